# revision 13
# baseline (speedup 1.0000x reference)
"""MoE Trainium2 kernel: multi-segment SPMD load balancing, all-bf16.

Every core runs the SAME program shape: S token segments of fixed sizes
(s_0..s_{S-1}), each segment paired with its own W1/W2 weight inputs (the
DATA differs per core, the shape doesn't — SPMD). Experts whose routed
count exceeds a segment are split across cores; light experts share cores.
A small exact solver (DP over segment-count compositions) picks segment
sizes minimizing the per-core column budget B = sum(sizes): for this
problem's routing it finds B=1040 vs 1129 for the busiest single expert
(ideal sum/8 = 1024).

Device math per core (per segment s, all bf16 operands, f32 PSUM):
    hT_s = relu(W1_s.T @ xeT_s)   (F, s_i)  one SBUF tile [128, NF, B]
    yT_s = W2_s.T @ hT_s          (D, s_i)  W2 128x128 chunks stationary,
                                            hT moving -> PE cost scales with
                                            the exact B (no 128 rounding)

bf16 rationale (measured on this HW): a bf16 matmul streams 512 cols in
216ns vs fp32r's 227ns (fp32r pays ~11ns/matmul extra on its stationary
4-byte load), has no fp32r >=256-col restriction, and halves DMA/SBUF.
End-to-end bf16 error here is 3.7e-3 vs the 2e-2 gate. fp8 (DoubleRow,
measured 2x MACs max) cannot meet 2e-2 without hi/lo splitting, which
costs 1.5x fp32r — rejected on measurement.

Host does routing, gather, gain folding (gain >= 0 so it commutes with
relu), weight packing, and the final transpose + scatter-add. b1/b2 are
zero in this problem; host-side fallback handles nonzero b2.
"""

import functools
import itertools
import math
import sys

import numpy as np

for _p in ("/opt/trn_rl_repo",):
    if _p not in sys.path:
        sys.path.append(_p)

T, E, D, F, C, K = 4096, 8, 1024, 4096, 1536, 2
N_CORES = 8
P = 128
KO = D // P
NF = F // P
NDC = D // P

_PROGRAMS = {}


def _solve_sizes(n_e, S):
    """Min-B segment sizes for S segments/core: 8 segments of each size
    class; every expert covered by exactly S segments (composition over
    classes, column sums == 8). Returns (sizes, comps) or None; comps[i]
    is the composition for the i-th largest expert."""
    nd = sorted(n_e, reverse=True)
    comps = [c for c in itertools.product(range(S + 1), repeat=S) if sum(c) == S]

    def feasible(sizes):
        @functools.lru_cache(maxsize=None)
        def rec(i, rem):
            if i == E:
                return () if all(r == 0 for r in rem) else None
            for c in comps:
                nr = tuple(r - cc for r, cc in zip(rem, c))
                if min(nr) < 0:
                    continue
                if sum(cc * s for cc, s in zip(c, sizes)) < nd[i]:
                    continue
                sub = rec(i + 1, nr)
                if sub is not None:
                    return (c,) + sub
            return None

        return rec(0, (8,) * S)

    def gen(prefix, rem, maxv, k):
        if k == 1:
            if 16 <= rem <= maxv:
                yield prefix + (rem,)
            return
        for v in range(min(maxv, rem - 16 * (k - 1)), 15, -8):
            yield from gen(prefix + (v,), rem - v, v, k - 1)

    lo = (sum(nd) + 8 * 8 - 1) // (8 * 8) * 8
    maxv = 512 if S >= 3 else 1024
    for B in range(lo, lo + 257, 8):
        for sizes in gen((), B, maxv, S):
            if sum((s + 511) // 512 for s in sizes) > 4:
                continue
            asg = feasible(sizes)
            if asg is not None:
                return sizes, asg
    return None


def _plan_segments(n_e):
    """Returns (sizes, seg_lists): seg_lists[s] = 8 x (expert, src_off, cnt)."""
    best = None
    for S in (3, 2):
        r = _solve_sizes(n_e, S)
        if r is not None and (best is None or sum(r[0]) < sum(best[0])):
            best = r
    if best is None:  # degenerate fallback: one expert per core, exact cap
        cap = int(max(n_e))
        return (cap,), [[(e, 0, int(n_e[e])) for e in range(E)]]
    sizes, comps = best
    S = len(sizes)
    ns = sorted(range(E), key=lambda e: -n_e[e])
    seg_lists = [[] for _ in range(S)]
    off = {e: 0 for e in range(E)}
    for rank, e in enumerate(ns):
        for s in range(S):
            for _ in range(comps[rank][s]):
                cnt = min(sizes[s], int(n_e[e]) - off[e])
                cnt = max(0, cnt)
                seg_lists[s].append((e, off[e], cnt))
                off[e] += cnt
    assert all(len(sl) == N_CORES for sl in seg_lists)
    assert all(off[e] == int(n_e[e]) for e in range(E))
    return sizes, seg_lists


def _seg_chunks(width, base):
    out = []
    rem, c0 = width, base
    while rem > 0:
        take = min(512, rem)
        out.append((c0, take))
        c0 += take
        rem -= take
    out.sort(key=lambda t: t[1])
    return out


def _build_program(sizes):
    import concourse.mybir as mybir
    import concourse.tile as tile
    from concourse import bacc

    f32 = mybir.dt.float32
    bf16 = mybir.dt.bfloat16
    Relu = mybir.ActivationFunctionType.Relu
    Ident = mybir.ActivationFunctionType.Identity

    S = len(sizes)
    B = sum(sizes)
    bases = [sum(sizes[:s]) for s in range(S)]
    # flat chunk list: (segment, c0, cw), smallest chunk first per segment
    flat = []
    for s in range(S):
        flat.extend((s, c0, cw) for (c0, cw) in _seg_chunks(sizes[s], bases[s]))
    # globally order smallest-first so every trailing matmul is wide enough
    # to hide the next stationary LDWEIGHTS
    flat.sort(key=lambda t: t[2])
    NCH = len(flat)
    assert NCH <= 4, f"too many PSUM tags: {NCH}"

    nc = bacc.Bacc(None, target_bir_lowering=False, debug=False)

    with tile.TileContext(nc) as tc:
        with tc.tile_pool(name="dram", bufs=1, space="DRAM") as dram:
            w1s = [
                dram.tile((NF, P, KO, P), bf16, kind="ExternalInput", name=f"w1{s}")
                for s in range(S)
            ]
            # w2 packed per d-chunk: (NDC, P, NF, 128)
            w2s = [
                dram.tile((NDC, P, NF, P), bf16, kind="ExternalInput", name=f"w2{s}")
                for s in range(S)
            ]
            xeT = dram.tile((D, B), bf16, kind="ExternalInput", name="xeT")
            yT = dram.tile((D, B), bf16, kind="ExternalOutput", name="yT")

        xeT_r = xeT[:].rearrange("(ko ki) c -> ki ko c", ki=P)

        with (
            tc.tile_pool(name="const", bufs=1) as constp,
            tc.tile_pool(name="xe", bufs=1) as xep,
            tc.tile_pool(name="ht", bufs=1) as htp,
            tc.tile_pool(name="ysb", bufs=1) as yp,
            tc.tile_pool(name="w1t", bufs=3 * S) as w1p,
            tc.tile_pool(name="w2t", bufs=2 * S) as w2p,
            tc.tile_pool(name="ps", bufs=2, space="PSUM") as psp,
        ):
            zero = constp.tile([P, 1], f32)
            nc.any.memset(zero[:], 0.0)

            # HAM warm-up: dependency-free matmuls so the PE clock ramps
            # while the first tiles stream in
            warm_w = constp.tile([P, P], bf16)
            nc.any.memset(warm_w[:], 0.0)
            warm_sb = constp.tile([P, 512], bf16)
            nc.any.memset(warm_sb[:], 0.0)
            warm_out = constp.tile([P, 1], f32)
            with tc.tile_pool(name="warmps", bufs=1, space="PSUM") as warmp:
                warm_ps = warmp.tile([P, 512], f32)
                for i in range(8):
                    nc.tensor.matmul(
                        warm_ps[:], warm_w[:], warm_sb[:], start=True, stop=True
                    )
                nc.vector.tensor_copy(warm_out[:], warm_ps[:, :1])

            # first stationary tiles head their DMA queues
            w1_first = [w1p.tile([P, KO, P], bf16, name="w1_t") for _ in range(S)]
            for s in range(S):
                nc.sync.dma_start(w1_first[s][:], w1s[s][0])

            xe_sb = xep.tile([P, KO, B], bf16)
            for ko in range(KO):
                nc.sync.dma_start(xe_sb[:, ko, :], xeT_r[:, ko, :])

            hT = htp.tile([P, NF, B], bf16)
            yT_sb = yp.tile([P, NDC, B], bf16)

            # W2 d-chunk tiles stream; the first two d-chunks prefetch during
            # the tail of MM1 (behind the w1 tiles they'd contend with)
            w2_t = {}
            pre = [(dh, s) for dh in range(2) for s in range(S)]

            # ---- MM1: hT = relu(W1_s.T @ xeT_s) ----
            for fc in range(NF):
                w1_t = []
                for s in range(S):
                    if fc == 0:
                        w1_t.append(w1_first[s])
                    else:
                        t = w1p.tile([P, KO, P], bf16, name="w1_t")
                        nc.sync.dma_start(t[:], w1s[s][fc])
                        w1_t.append(t)
                if fc >= NF - len(pre):
                    dh, s = pre[fc - (NF - len(pre))]
                    w2_t[(dh, s)] = w2p.tile([P, NF, P], bf16, name="w2_t")
                    nc.sync.dma_start(w2_t[(dh, s)][:], w2s[s][dh])
                ph = {
                    i: psp.tile([P, cw], f32, name=f"p{i}", tag=f"p{i}")
                    for i, (s, c0, cw) in enumerate(flat)
                }
                for k in range(KO):
                    for i, (s, c0, cw) in enumerate(flat):
                        nc.tensor.matmul(
                            ph[i][:],
                            w1_t[s][:, k, :],
                            xe_sb[:, k, c0 : c0 + cw],
                            start=(k == 0),
                            stop=(k == KO - 1),
                        )
                for i, (s, c0, cw) in enumerate(flat):
                    nc.scalar.activation(
                        hT[:, fc, c0 : c0 + cw], ph[i][:], Relu, bias=zero[:]
                    )

            # ---- MM2: yT = W2_s.T @ hT_s ----
            for dh in range(NDC):
                if dh + 2 < NDC:
                    for s in range(S):
                        w2_t[(dh + 2, s)] = w2p.tile([P, NF, P], bf16, name="w2_t")
                        nc.sync.dma_start(w2_t[(dh + 2, s)][:], w2s[s][dh + 2])
                py = {
                    i: psp.tile([P, cw], f32, name=f"py{i}", tag=f"p{i}")
                    for i, (s, c0, cw) in enumerate(flat)
                }
                for fs in range(NF):
                    for i, (s, c0, cw) in enumerate(flat):
                        nc.tensor.matmul(
                            py[i][:],
                            w2_t[(dh, s)][:, fs, :],
                            hT[:, fs, c0 : c0 + cw],
                            start=(fs == 0),
                            stop=(fs == NF - 1),
                        )
                for i, (s, c0, cw) in enumerate(flat):
                    dst = yT_sb[:, dh, c0 : c0 + cw]
                    if i % 2 == 0:
                        nc.vector.tensor_copy(dst, py[i][:])
                    else:
                        nc.scalar.activation(dst, py[i][:], Ident, bias=zero[:])
                    nc.sync.dma_start(yT[dh * P : (dh + 1) * P, c0 : c0 + cw], dst)

    nc.compile()
    names = dict(
        w1=[t.name for t in w1s],
        w2=[t.name for t in w2s],
        xeT=xeT.name,
        y=yT.name,
    )
    return nc, names


def _get_program(sizes):
    if sizes not in _PROGRAMS:
        _PROGRAMS[sizes] = _build_program(sizes)
    return _PROGRAMS[sizes]


# test.py can set RUN_KWARGS (e.g. dict(trace=True)) and read LAST_RESULTS
RUN_KWARGS = {}
LAST_RESULTS = None


def kernel(x, route_mask, route_weight, W1, b1, W2, b2):
    import ml_dtypes

    from concourse.bass_utils import run_bass_kernel_spmd

    global LAST_RESULTS

    bf = ml_dtypes.bfloat16

    x = np.asarray(x, dtype=np.float32)
    route_mask = np.asarray(route_mask, dtype=bool)
    route_weight = np.asarray(route_weight, dtype=np.float32)
    W1 = np.asarray(W1, dtype=np.float32)
    W2 = np.asarray(W2, dtype=np.float32)
    b1 = np.asarray(b1, dtype=np.float32)
    b2 = np.asarray(b2, dtype=np.float32)
    if np.any(b1):
        raise NotImplementedError("nonzero b1 not supported")

    # --- routing: per-expert top-C tokens by route weight ---
    w_et = np.where(route_mask.T, route_weight.T, -np.inf)  # (E, T)
    order = np.argsort(-w_et, axis=1, kind="stable")[:, :C]  # (E, C)
    vals = np.take_along_axis(w_et, order, axis=1)
    valid = np.isfinite(vals)
    gain = np.where(valid, vals, 0.0).astype(np.float32)

    n_e = np.minimum(valid.sum(axis=1), C).astype(int)
    sizes, seg_lists = _plan_segments(n_e)
    S = len(sizes)
    B = sum(sizes)
    bases = [sum(sizes[:s]) for s in range(S)]

    nc, names = _get_program(sizes)

    # pre-pack per-expert weights once (an expert may appear on 2+ cores)
    used = sorted({e for sl in seg_lists for e, _, cnt in sl if cnt > 0})
    w1p_, w2p_ = {}, {}
    for e in used:
        w1p_[e] = np.ascontiguousarray(
            W1[e].reshape(KO, P, NF, P).transpose(2, 1, 0, 3).astype(bf)
        )
        w2p_[e] = np.ascontiguousarray(
            W2[e].reshape(NF, P, NDC, P).transpose(2, 1, 0, 3).astype(bf)
        )
    w1_pad = np.zeros((NF, P, KO, P), bf)
    w2_pad = np.zeros((NDC, P, NF, P), bf)

    in_maps = []
    for core in range(N_CORES):
        xeT_np = np.zeros((D, B), bf)
        im = {}
        for s in range(S):
            e, off, cnt = seg_lists[s][core]
            if cnt > 0:
                idx = order[e, off : off + cnt]
                xe = x[idx] * gain[e, off : off + cnt][:, None]
                xeT_np[:, bases[s] : bases[s] + cnt] = xe.T.astype(bf)
                im[names["w1"][s]] = w1p_[e]
                im[names["w2"][s]] = w2p_[e]
            else:
                im[names["w1"][s]] = w1_pad
                im[names["w2"][s]] = w2_pad
        im[names["xeT"]] = np.ascontiguousarray(xeT_np)
        in_maps.append(im)

    res = run_bass_kernel_spmd(nc, in_maps, list(range(N_CORES)), **RUN_KWARGS)
    LAST_RESULTS = res

    # --- combine: transpose + scatter-add ---
    y = np.zeros((T, D), np.float32)
    for core in range(N_CORES):
        yTc = res.results[core][names["y"]]
        for s in range(S):
            e, off, cnt = seg_lists[s][core]
            if cnt == 0:
                continue
            ye = yTc[:, bases[s] : bases[s] + cnt].T.astype(np.float32)
            if np.any(b2):
                ye = ye + gain[e, off : off + cnt][:, None] * b2[e][None, :]
            y[order[e, off : off + cnt]] += ye
    return y


# revision 18
# speedup vs baseline: 1.0162x; 1.0162x over previous
"""MoE Trainium2 kernel v4: two-segment SPMD load balancing, all-bf16.

Every core runs the SAME program shape: two token segments of sizes (a, b),
each segment paired with its own W1/W2 weight inputs. Heavy experts (whose
routed count exceeds the segment budget) are split across two cores'
a-segments; light experts pair up in b-segments. The (a, b) sizes are chosen
per-input by a tiny solver (k experts split a+a, 8-2k run a+b, k pair b+b)
minimizing a+b — for balanced routing this lands ~4% above the perfect
sum/8 split vs ~11% for one-expert-per-core.

Device math per core (both segments, bf16):
    hT = relu(W1_s.T @ xeT_s)   (F, a|b)  in one SBUF tile [128, NF, a+b]
    yT = W2_s.T @ hT_s          (D, a|b)  W2 128x128 chunks stationary

W2 is host-packed per d-chunk (NDC, P, NF, 128) and streamed during MM2;
W1 is host-packed (NF, P, KO, P) and streamed during MM1.
"""

import math
import sys

import numpy as np

for _p in ("/opt/trn_rl_repo",):
    if _p not in sys.path:
        sys.path.append(_p)

T, E, D, F, C, K = 4096, 8, 1024, 4096, 1536, 2
N_CORES = 8
P = 128
KO = D // P
NF = F // P
NDC = D // P

_PROGRAMS = {}


def _plan_segments(n_e):
    """Pick segment sizes (a, b) and assign experts to the 8 a-segs + 8 b-segs.

    Returns (a, b, a_segs, b_segs) where each seg list has 8 entries
    (expert, src_off, cnt): the segment holds slots [src_off, src_off+cnt)
    of that expert's gain-sorted slot list (cnt may be 0 for pad segments).
    """
    ns = sorted(range(E), key=lambda e: -n_e[e])  # experts by load desc
    best = None
    for k in range(0, E // 2 + 1):
        top = [n_e[e] for e in ns[:k]]
        mid = [n_e[e] for e in ns[k : E - k]]
        bot = [n_e[e] for e in ns[E - k :]]
        a_min = max([(v + 1) // 2 for v in top], default=0)
        b_min = max([(v + 1) // 2 for v in bot], default=0)
        mid_max = max(mid, default=0)
        a = max(a_min, (mid_max + 1) // 2, 16)
        b = max(b_min, mid_max - a, 16)
        a = (a + 7) // 8 * 8
        b = (b + 7) // 8 * 8
        if best is None or a + b < best[0] + best[1]:
            best = (a, b, k)
    a, b, k = best
    a_segs, b_segs = [], []
    for i, e in enumerate(ns):
        n = n_e[e]
        if i < k:  # a + a
            a_segs.append((e, 0, min(a, n)))
            a_segs.append((e, min(a, n), max(0, n - a)))
        elif i < E - k:  # a + b
            a_segs.append((e, 0, min(a, n)))
            b_segs.append((e, min(a, n), max(0, n - a)))
        else:  # b + b
            b_segs.append((e, 0, min(b, n)))
            b_segs.append((e, min(b, n), max(0, n - b)))
    assert len(a_segs) == N_CORES and len(b_segs) == N_CORES
    return a, b, a_segs, b_segs


def _seg_chunks(width, base):
    """<=512-wide chunk list for one segment, smallest chunk first."""
    out = []
    rem, c0 = width, base
    while rem > 0:
        take = min(512, rem)
        out.append((c0, take))
        c0 += take
        rem -= take
    out.sort(key=lambda t: t[1])
    return out


def _build_program(a, b):
    import concourse.mybir as mybir
    import concourse.tile as tile
    from concourse import bacc

    f32 = mybir.dt.float32
    bf16 = mybir.dt.bfloat16
    Relu = mybir.ActivationFunctionType.Relu
    Ident = mybir.ActivationFunctionType.Identity

    B = a + b
    # per-segment chunk lists; global tag numbering across both
    seg_chunks = [_seg_chunks(a, 0), _seg_chunks(b, a)]
    flat = [(s, c0, cw) for s in (0, 1) for (c0, cw) in seg_chunks[s]]

    nc = bacc.Bacc(None, target_bir_lowering=False, debug=False)

    with tile.TileContext(nc) as tc:
        with tc.tile_pool(name="dram", bufs=1, space="DRAM") as dram:
            w1s = [
                dram.tile((NF, P, KO, P), bf16, kind="ExternalInput", name=f"w1{s}")
                for s in range(2)
            ]
            # w2 packed per d-chunk: (NDC, P, NF, 128)
            w2s = [
                dram.tile((NDC, P, NF, P), bf16, kind="ExternalInput", name=f"w2{s}")
                for s in range(2)
            ]
            xeT = dram.tile((D, B), bf16, kind="ExternalInput", name="xeT")
            yT = dram.tile((D, B), bf16, kind="ExternalOutput", name="yT")

        xeT_r = xeT[:].rearrange("(ko ki) c -> ki ko c", ki=P)

        with (
            tc.tile_pool(name="const", bufs=1) as constp,
            tc.tile_pool(name="xe", bufs=1) as xep,
            tc.tile_pool(name="ht", bufs=1) as htp,
            tc.tile_pool(name="ysb", bufs=1) as yp,
            tc.tile_pool(name="w1t", bufs=12) as w1p,
            tc.tile_pool(name="w2t", bufs=6) as w2p,
            tc.tile_pool(name="ps", bufs=2, space="PSUM") as psp,
        ):
            zero = constp.tile([P, 1], f32)
            nc.any.memset(zero[:], 0.0)

            warm_w = constp.tile([P, P], bf16)
            nc.any.memset(warm_w[:], 0.0)
            warm_sb = constp.tile([P, 512], bf16)
            nc.any.memset(warm_sb[:], 0.0)
            warm_out = constp.tile([P, 1], f32)
            with tc.tile_pool(name="warmps", bufs=1, space="PSUM") as warmp:
                warm_ps = warmp.tile([P, 512], f32)
                for i in range(8):
                    nc.tensor.matmul(
                        warm_ps[:], warm_w[:], warm_sb[:], start=True, stop=True
                    )
                nc.vector.tensor_copy(warm_out[:], warm_ps[:, :1])

            # first stationary tiles head their DMA queues
            w1_first = [w1p.tile([P, KO, P], bf16, name="w1_t") for s in range(2)]
            for s in range(2):
                nc.sync.dma_start(w1_first[s][:], w1s[s][0])

            # xe in 4 ko-pair transfers: few sync-queue issues (~600ns each)
            # while the first matmul only waits on the first 532KB
            xe_sb = xep.tile([P, KO, B], bf16)
            for kp in range(KO // 2):
                nc.sync.dma_start(
                    xe_sb[:, 2 * kp : 2 * kp + 2, :], xeT_r[:, 2 * kp : 2 * kp + 2, :]
                )

            hT = htp.tile([P, NF, B], bf16)
            yT_sb = yp.tile([P, NDC, B], bf16)

            # W2 d-chunk tiles are streamed; first two d-chunks prefetch
            # during the tail of MM1 (after the w1 tiles they'd contend with)
            w2_t = {}

            # ---- MM1 ----
            for fc in range(NF):
                w1_t = []
                for s in range(2):
                    if fc == 0:
                        w1_t.append(w1_first[s])
                    else:
                        t = w1p.tile([P, KO, P], bf16, name="w1_t")
                        nc.sync.dma_start(t[:], w1s[s][fc])
                        w1_t.append(t)
                if fc >= NF - 6:
                    dh, s = divmod(fc - (NF - 6), 2)
                    w2_t[(dh, s)] = w2p.tile([P, NF, P], bf16, name="w2_t")
                    nc.sync.dma_start(w2_t[(dh, s)][:], w2s[s][dh])
                ph = {
                    i: psp.tile([P, cw], f32, name=f"p{i}", tag=f"p{i}")
                    for i, (s, c0, cw) in enumerate(flat)
                }
                for k in range(KO):
                    for i, (s, c0, cw) in enumerate(flat):
                        nc.tensor.matmul(
                            ph[i][:],
                            w1_t[s][:, k, :],
                            xe_sb[:, k, c0 : c0 + cw],
                            start=(k == 0),
                            stop=(k == KO - 1),
                        )
                for i, (s, c0, cw) in enumerate(flat):
                    nc.scalar.activation(
                        hT[:, fc, c0 : c0 + cw], ph[i][:], Relu, bias=zero[:]
                    )

            # ---- MM2 ----
            for dh in range(NDC):
                if dh + 3 < NDC:
                    for s in range(2):
                        w2_t[(dh + 3, s)] = w2p.tile([P, NF, P], bf16, name="w2_t")
                        nc.sync.dma_start(w2_t[(dh + 3, s)][:], w2s[s][dh + 3])
                py = {
                    i: psp.tile([P, cw], f32, name=f"py{i}", tag=f"p{i}")
                    for i, (s, c0, cw) in enumerate(flat)
                }
                for fs in range(NF):
                    for i, (s, c0, cw) in enumerate(flat):
                        nc.tensor.matmul(
                            py[i][:],
                            w2_t[(dh, s)][:, fs, :],
                            hT[:, fs, c0 : c0 + cw],
                            start=(fs == 0),
                            stop=(fs == NF - 1),
                        )
                for i, (s, c0, cw) in enumerate(flat):
                    dst = yT_sb[:, dh, c0 : c0 + cw]
                    if i % 2 == 0:
                        nc.vector.tensor_copy(dst, py[i][:])
                    else:
                        nc.scalar.activation(dst, py[i][:], Ident, bias=zero[:])
                    nc.sync.dma_start(yT[dh * P : (dh + 1) * P, c0 : c0 + cw], dst)

    nc.compile()
    names = dict(
        w1=[t.name for t in w1s],
        w2=[t.name for t in w2s],
        xeT=xeT.name,
        y=yT.name,
    )
    return nc, names


def _get_program(a, b):
    if (a, b) not in _PROGRAMS:
        _PROGRAMS[(a, b)] = _build_program(a, b)
    return _PROGRAMS[(a, b)]


RUN_KWARGS = {}
LAST_RESULTS = None


def kernel(x, route_mask, route_weight, W1, b1, W2, b2):
    import ml_dtypes

    from concourse.bass_utils import run_bass_kernel_spmd

    global LAST_RESULTS

    bf = ml_dtypes.bfloat16

    x = np.asarray(x, dtype=np.float32)
    route_mask = np.asarray(route_mask, dtype=bool)
    route_weight = np.asarray(route_weight, dtype=np.float32)
    W1 = np.asarray(W1, dtype=np.float32)
    W2 = np.asarray(W2, dtype=np.float32)
    b1 = np.asarray(b1, dtype=np.float32)
    b2 = np.asarray(b2, dtype=np.float32)
    if np.any(b1):
        raise NotImplementedError("nonzero b1 not supported")

    w_et = np.where(route_mask.T, route_weight.T, -np.inf)  # (E, T)
    order = np.argsort(-w_et, axis=1, kind="stable")[:, :C]  # (E, C)
    vals = np.take_along_axis(w_et, order, axis=1)
    valid = np.isfinite(vals)
    gain = np.where(valid, vals, 0.0).astype(np.float32)

    n_e = np.minimum(valid.sum(axis=1), C).astype(int)
    a, b, a_segs, b_segs = _plan_segments(n_e)
    B = a + b

    nc, names = _get_program(a, b)

    # pre-pack per-expert weights once (an expert may appear on 2 cores)
    used = sorted({e for e, _, cnt in a_segs + b_segs if cnt > 0})
    w1p_, w2p_ = {}, {}
    for e in used:
        w1p_[e] = np.ascontiguousarray(
            W1[e].reshape(KO, P, NF, P).transpose(2, 1, 0, 3).astype(bf)
        )
        w2p_[e] = np.ascontiguousarray(
            W2[e].reshape(NF, P, NDC, P).transpose(2, 1, 0, 3).astype(bf)
        )
    w1_pad = np.zeros((NF, P, KO, P), bf)
    w2_pad = np.zeros((NDC, P, NF, P), bf)

    in_maps = []
    for core in range(N_CORES):
        segs = [(a_segs[core], 0, a), (b_segs[core], a, b)]
        xeT_np = np.zeros((D, B), bf)
        im = {}
        for s, ((e, off, cnt), base, width) in enumerate(segs):
            if cnt > 0:
                idx = order[e, off : off + cnt]
                xe = x[idx] * gain[e, off : off + cnt][:, None]
                xeT_np[:, base : base + cnt] = xe.T.astype(bf)
                im[names["w1"][s]] = w1p_[e]
                im[names["w2"][s]] = w2p_[e]
            else:
                im[names["w1"][s]] = w1_pad
                im[names["w2"][s]] = w2_pad
        im[names["xeT"]] = np.ascontiguousarray(xeT_np)
        in_maps.append(im)

    res = run_bass_kernel_spmd(nc, in_maps, list(range(N_CORES)), **RUN_KWARGS)
    LAST_RESULTS = res

    y = np.zeros((T, D), np.float32)
    for core in range(N_CORES):
        yTc = res.results[core][names["y"]]
        for (e, off, cnt), base, width in (
            (a_segs[core], 0, a),
            (b_segs[core], a, b),
        ):
            if cnt == 0:
                continue
            ye = yTc[:, base : base + cnt].T.astype(np.float32)
            if np.any(b2):
                ye = ye + gain[e, off : off + cnt][:, None] * b2[e][None, :]
            y[order[e, off : off + cnt]] += ye
    return y


# revision 22
# speedup vs baseline: 1.0168x; 1.0005x over previous
"""MoE Trainium2 kernel v4: two-segment SPMD load balancing, all-bf16.

Every core runs the SAME program shape: two token segments of sizes (a, b),
each segment paired with its own W1/W2 weight inputs. Heavy experts (whose
routed count exceeds the segment budget) are split across two cores'
a-segments; light experts pair up in b-segments. The (a, b) sizes are chosen
per-input by a tiny solver (k experts split a+a, 8-2k run a+b, k pair b+b)
minimizing a+b — for balanced routing this lands ~4% above the perfect
sum/8 split vs ~11% for one-expert-per-core.

Device math per core (both segments, bf16):
    hT = relu(W1_s.T @ xeT_s)   (F, a|b)  in one SBUF tile [128, NF, a+b]
    yT = W2_s.T @ hT_s          (D, a|b)  W2 128x128 chunks stationary

W2 is host-packed per d-chunk (NDC, P, NF, 128) and streamed during MM2;
W1 is host-packed (NF, P, KO, P) and streamed during MM1.
"""

import math
import sys

import numpy as np

for _p in ("/opt/trn_rl_repo",):
    if _p not in sys.path:
        sys.path.append(_p)

T, E, D, F, C, K = 4096, 8, 1024, 4096, 1536, 2
N_CORES = 8
P = 128
KO = D // P
NF = F // P
NDC = D // P

_PROGRAMS = {}


def _plan_segments(n_e):
    """Pick segment sizes (a, b) and assign experts to the 8 a-segs + 8 b-segs.

    Returns (a, b, a_segs, b_segs) where each seg list has 8 entries
    (expert, src_off, cnt): the segment holds slots [src_off, src_off+cnt)
    of that expert's gain-sorted slot list (cnt may be 0 for pad segments).
    """
    ns = sorted(range(E), key=lambda e: -n_e[e])  # experts by load desc
    best = None
    for k in range(0, E // 2 + 1):
        top = [n_e[e] for e in ns[:k]]
        mid = [n_e[e] for e in ns[k : E - k]]
        bot = [n_e[e] for e in ns[E - k :]]
        a_min = max([(v + 1) // 2 for v in top], default=0)
        b_min = max([(v + 1) // 2 for v in bot], default=0)
        mid_max = max(mid, default=0)
        a = max(a_min, (mid_max + 1) // 2, 16)
        b = max(b_min, mid_max - a, 16)
        a = (a + 7) // 8 * 8
        b = (b + 7) // 8 * 8
        if best is None or a + b < best[0] + best[1]:
            best = (a, b, k)
    a, b, k = best
    a_segs, b_segs = [], []
    for i, e in enumerate(ns):
        n = n_e[e]
        if i < k:  # a + a
            a_segs.append((e, 0, min(a, n)))
            a_segs.append((e, min(a, n), max(0, n - a)))
        elif i < E - k:  # a + b
            a_segs.append((e, 0, min(a, n)))
            b_segs.append((e, min(a, n), max(0, n - a)))
        else:  # b + b
            b_segs.append((e, 0, min(b, n)))
            b_segs.append((e, min(b, n), max(0, n - b)))
    assert len(a_segs) == N_CORES and len(b_segs) == N_CORES
    return a, b, a_segs, b_segs


def _seg_chunks(width, base):
    """<=512-wide chunk list for one segment, smallest chunk first."""
    out = []
    rem, c0 = width, base
    while rem > 0:
        take = min(512, rem)
        out.append((c0, take))
        c0 += take
        rem -= take
    out.sort(key=lambda t: t[1])
    return out


def _build_program(a, b):
    import concourse.mybir as mybir
    import concourse.tile as tile
    from concourse import bacc

    f32 = mybir.dt.float32
    bf16 = mybir.dt.bfloat16
    Relu = mybir.ActivationFunctionType.Relu
    Ident = mybir.ActivationFunctionType.Identity

    B = a + b
    # per-segment chunk lists; global tag numbering across both
    seg_chunks = [_seg_chunks(a, 0), _seg_chunks(b, a)]
    flat = [(s, c0, cw) for s in (0, 1) for (c0, cw) in seg_chunks[s]]

    nc = bacc.Bacc(None, target_bir_lowering=False, debug=False)

    with tile.TileContext(nc) as tc:
        with tc.tile_pool(name="dram", bufs=1, space="DRAM") as dram:
            w1s = [
                dram.tile((NF, P, KO, P), bf16, kind="ExternalInput", name=f"w1{s}")
                for s in range(2)
            ]
            # w2 packed per d-chunk: (NDC, P, NF, 128)
            w2s = [
                dram.tile((NDC, P, NF, P), bf16, kind="ExternalInput", name=f"w2{s}")
                for s in range(2)
            ]
            xeT = dram.tile((D, B), bf16, kind="ExternalInput", name="xeT")
            yT = dram.tile((D, B), bf16, kind="ExternalOutput", name="yT")

        xeT_r = xeT[:].rearrange("(ko ki) c -> ki ko c", ki=P)

        with (
            tc.tile_pool(name="const", bufs=1) as constp,
            tc.tile_pool(name="xe", bufs=1) as xep,
            tc.tile_pool(name="ht", bufs=1) as htp,
            tc.tile_pool(name="ysb", bufs=1) as yp,
            tc.tile_pool(name="w1t", bufs=8) as w1p,
            tc.tile_pool(name="w2t", bufs=4) as w2p,
            tc.tile_pool(name="ps", bufs=2, space="PSUM") as psp,
        ):
            zero = constp.tile([P, 1], f32)
            nc.any.memset(zero[:], 0.0)

            warm_w = constp.tile([P, P], bf16)
            nc.any.memset(warm_w[:], 0.0)
            warm_sb = constp.tile([P, 512], bf16)
            nc.any.memset(warm_sb[:], 0.0)
            warm_out = constp.tile([P, 1], f32)
            with tc.tile_pool(name="warmps", bufs=1, space="PSUM") as warmp:
                warm_ps = warmp.tile([P, 512], f32)
                for i in range(6):
                    nc.tensor.matmul(
                        warm_ps[:], warm_w[:], warm_sb[:], start=True, stop=True
                    )
                nc.vector.tensor_copy(warm_out[:], warm_ps[:, :1])

            # first-matmul dependencies head the DMA issue order: xe[k0]
            # first (every chunk of fc0/k0 reads it), then the two first
            # stationary tiles; each dma_start costs ~600ns of sync-queue
            # issue time so order matters at startup
            xe_sb = xep.tile([P, KO, B], bf16)
            nc.sync.dma_start(xe_sb[:, 0, :], xeT_r[:, 0, :])
            w1_first = [w1p.tile([P, KO, P], bf16, name="w1_t") for s in range(2)]
            for s in range(2):
                nc.sync.dma_start(w1_first[s][:], w1s[s][0])
            for ko in range(1, KO):
                nc.sync.dma_start(xe_sb[:, ko, :], xeT_r[:, ko, :])

            hT = htp.tile([P, NF, B], bf16)
            yT_sb = yp.tile([P, NDC, B], bf16)

            # W2 d-chunk tiles are streamed; first two d-chunks prefetch
            # during the tail of MM1 (after the w1 tiles they'd contend with)
            w2_t = {}

            # ---- MM1 ----
            for fc in range(NF):
                w1_t = []
                for s in range(2):
                    if fc == 0:
                        w1_t.append(w1_first[s])
                    else:
                        t = w1p.tile([P, KO, P], bf16, name="w1_t")
                        nc.sync.dma_start(t[:], w1s[s][fc])
                        w1_t.append(t)
                if fc >= NF - 4:
                    dh, s = divmod(fc - (NF - 4), 2)
                    w2_t[(dh, s)] = w2p.tile([P, NF, P], bf16, name="w2_t")
                    nc.sync.dma_start(w2_t[(dh, s)][:], w2s[s][dh])
                ph = {
                    i: psp.tile([P, cw], f32, name=f"p{i}", tag=f"p{i}")
                    for i, (s, c0, cw) in enumerate(flat)
                }
                for k in range(KO):
                    for i, (s, c0, cw) in enumerate(flat):
                        nc.tensor.matmul(
                            ph[i][:],
                            w1_t[s][:, k, :],
                            xe_sb[:, k, c0 : c0 + cw],
                            start=(k == 0),
                            stop=(k == KO - 1),
                        )
                for i, (s, c0, cw) in enumerate(flat):
                    nc.scalar.activation(
                        hT[:, fc, c0 : c0 + cw], ph[i][:], Relu, bias=zero[:]
                    )

            # ---- MM2 ----
            for dh in range(NDC):
                if dh + 2 < NDC:
                    for s in range(2):
                        w2_t[(dh + 2, s)] = w2p.tile([P, NF, P], bf16, name="w2_t")
                        nc.sync.dma_start(w2_t[(dh + 2, s)][:], w2s[s][dh + 2])
                py = {
                    i: psp.tile([P, cw], f32, name=f"py{i}", tag=f"p{i}")
                    for i, (s, c0, cw) in enumerate(flat)
                }
                for fs in range(NF):
                    for i, (s, c0, cw) in enumerate(flat):
                        nc.tensor.matmul(
                            py[i][:],
                            w2_t[(dh, s)][:, fs, :],
                            hT[:, fs, c0 : c0 + cw],
                            start=(fs == 0),
                            stop=(fs == NF - 1),
                        )
                for i, (s, c0, cw) in enumerate(flat):
                    dst = yT_sb[:, dh, c0 : c0 + cw]
                    if i == len(flat) - 1:
                        # last chunk is on the critical path after the final
                        # stop-matmul: split the cast across both engines
                        h1 = cw // 2
                        nc.vector.tensor_copy(dst[:, :h1], py[i][:, :h1])
                        nc.scalar.activation(
                            dst[:, h1:], py[i][:, h1:], Ident, bias=zero[:]
                        )
                    elif i % 2 == 0:
                        nc.vector.tensor_copy(dst, py[i][:])
                    else:
                        nc.scalar.activation(dst, py[i][:], Ident, bias=zero[:])
                    nc.sync.dma_start(yT[dh * P : (dh + 1) * P, c0 : c0 + cw], dst)

    nc.compile()
    names = dict(
        w1=[t.name for t in w1s],
        w2=[t.name for t in w2s],
        xeT=xeT.name,
        y=yT.name,
    )
    return nc, names


def _get_program(a, b):
    if (a, b) not in _PROGRAMS:
        _PROGRAMS[(a, b)] = _build_program(a, b)
    return _PROGRAMS[(a, b)]


RUN_KWARGS = {}
LAST_RESULTS = None


def kernel(x, route_mask, route_weight, W1, b1, W2, b2):
    import ml_dtypes

    from concourse.bass_utils import run_bass_kernel_spmd

    global LAST_RESULTS

    bf = ml_dtypes.bfloat16

    x = np.asarray(x, dtype=np.float32)
    route_mask = np.asarray(route_mask, dtype=bool)
    route_weight = np.asarray(route_weight, dtype=np.float32)
    W1 = np.asarray(W1, dtype=np.float32)
    W2 = np.asarray(W2, dtype=np.float32)
    b1 = np.asarray(b1, dtype=np.float32)
    b2 = np.asarray(b2, dtype=np.float32)
    if np.any(b1):
        raise NotImplementedError("nonzero b1 not supported")

    w_et = np.where(route_mask.T, route_weight.T, -np.inf)  # (E, T)
    order = np.argsort(-w_et, axis=1, kind="stable")[:, :C]  # (E, C)
    vals = np.take_along_axis(w_et, order, axis=1)
    valid = np.isfinite(vals)
    gain = np.where(valid, vals, 0.0).astype(np.float32)

    n_e = np.minimum(valid.sum(axis=1), C).astype(int)
    a, b, a_segs, b_segs = _plan_segments(n_e)
    B = a + b

    nc, names = _get_program(a, b)

    # pre-pack per-expert weights once (an expert may appear on 2 cores)
    used = sorted({e for e, _, cnt in a_segs + b_segs if cnt > 0})
    w1p_, w2p_ = {}, {}
    for e in used:
        w1p_[e] = np.ascontiguousarray(
            W1[e].reshape(KO, P, NF, P).transpose(2, 1, 0, 3).astype(bf)
        )
        w2p_[e] = np.ascontiguousarray(
            W2[e].reshape(NF, P, NDC, P).transpose(2, 1, 0, 3).astype(bf)
        )
    w1_pad = np.zeros((NF, P, KO, P), bf)
    w2_pad = np.zeros((NDC, P, NF, P), bf)

    in_maps = []
    for core in range(N_CORES):
        segs = [(a_segs[core], 0, a), (b_segs[core], a, b)]
        xeT_np = np.zeros((D, B), bf)
        im = {}
        for s, ((e, off, cnt), base, width) in enumerate(segs):
            if cnt > 0:
                idx = order[e, off : off + cnt]
                xe = x[idx] * gain[e, off : off + cnt][:, None]
                xeT_np[:, base : base + cnt] = xe.T.astype(bf)
                im[names["w1"][s]] = w1p_[e]
                im[names["w2"][s]] = w2p_[e]
            else:
                im[names["w1"][s]] = w1_pad
                im[names["w2"][s]] = w2_pad
        im[names["xeT"]] = np.ascontiguousarray(xeT_np)
        in_maps.append(im)

    res = run_bass_kernel_spmd(nc, in_maps, list(range(N_CORES)), **RUN_KWARGS)
    LAST_RESULTS = res

    y = np.zeros((T, D), np.float32)
    for core in range(N_CORES):
        yTc = res.results[core][names["y"]]
        for (e, off, cnt), base, width in (
            (a_segs[core], 0, a),
            (b_segs[core], a, b),
        ):
            if cnt == 0:
                continue
            ye = yTc[:, base : base + cnt].T.astype(np.float32)
            if np.any(b2):
                ye = ye + gain[e, off : off + cnt][:, None] * b2[e][None, :]
            y[order[e, off : off + cnt]] += ye
    return y
